# revision 5
# baseline (speedup 1.0000x reference)
"""Bidirectional LSTM (B=32, T=2048, I=256, H=128/dir) for 8 Trainium2 cores.

v4: baseline block-Jacobi structure (4 streams, L=128 blocks, pos/neg W_hh
delta matmuls, hardware c-scan) with the work cut down:

 - K=3 sweeps instead of 5 (rel-l2 ~5e-3 vs fp32 reference; gate is 2e-2).
 - x / W_ih in bf16 (halves input DMA; PE still 1 col/cycle).
 - fp16 everywhere downstream (gates, h feedback, output; c-scan fp32).
 - zt+z fused into one scalar_tensor_tensor: z' = (sig(2g) - 0.5)*sig(i)
   = z/2, so the scan accumulates c/2 and tanh uses scale=2.

Cores 0-3 run the forward LSTM over 8 batch rows each; cores 4-7 run the
backward LSTM over host-flipped sequences.  Everything is gate-major:
SBUF/PSUM tiles are [128 partitions = gate/h element, cols = (batch-unit,
time) b-major].
"""

import numpy as np
import ml_dtypes

import concourse.bass as bass
import concourse.bacc as bacc
import concourse.tile as tile
from concourse import mybir
from concourse.bass_utils import run_bass_kernel_spmd

# Problem shapes (hardcoded per contract)
B, T, I, HS = 32, 2048, 256, 256
H = 128          # per-direction hidden
G4 = 4 * H       # 512 stacked gates
NCORES = 8
U = 8            # sequences per core
S = 4            # independent streams per core (pipelining)
BS = U // S      # sequences per stream
L = 128          # time-block length
NBLK = T // L
K_SWEEPS = 3
C = BS * L       # columns per stream-block (256)

# gate chunk order inside the 4*H dim: (i, f, o, g); reference order is (i, f, g, o)
PERM = [0, 1, 3, 2]

F32 = mybir.dt.float32
BF16 = mybir.dt.bfloat16
F16 = mybir.dt.float16
F32R = mybir.dt.float32r

_NC_CACHE = {}


def _build_nc():
    nc = bacc.Bacc()
    xt_h = nc.dram_tensor("xt", [2, 128, U * T], BF16, kind="ExternalInput")
    wih_h = nc.dram_tensor("wih", [2, 128, G4], BF16, kind="ExternalInput")
    whh_h = nc.dram_tensor("whh", [128, G4], F16, kind="ExternalInput")
    bias_h = nc.dram_tensor("bias", [1, G4], F32R, kind="ExternalInput")
    out_h = nc.dram_tensor("out", [128, U * T], F16, kind="ExternalOutput")

    sig = mybir.ActivationFunctionType.Sigmoid
    tanh = mybir.ActivationFunctionType.Tanh
    mult = mybir.AluOpType.mult
    add = mybir.AluOpType.add
    sub = mybir.AluOpType.subtract

    with tile.TileContext(nc) as tc:
        with (
            tc.tile_pool(name="singles", bufs=1) as singles,
            tc.tile_pool(name="work", bufs=3) as work,
            tc.tile_pool(name="psum", bufs=1, space="PSUM") as psump,
        ):
            # --- constants / weights ---
            wih_sb = singles.tile([128, 2, G4], BF16, tag="wih")
            nc.sync.dma_start(out=wih_sb, in_=wih_h[:, :, :].transpose([1, 0, 2]))
            whh_sb = singles.tile([128, G4], F16, tag="whh")
            nc.sync.dma_start(out=whh_sb, in_=whh_h[:, :])
            whh_neg = singles.tile([128, G4], F16, tag="whhn")
            nc.vector.tensor_scalar_mul(whh_neg, whh_sb, -1.0)
            bias_sb = singles.tile([1, G4], F32R, tag="bias")
            nc.sync.dma_start(out=bias_sb, in_=bias_h[:, :])
            ones_sb = singles.tile([1, C], F32R, tag="ones")
            nc.vector.memset(ones_sb.bitcast(mybir.dt.uint32), 0x3F800000)

            # Warm-up matmuls: consume every weight tile once so later
            # matmuls inherit the weight-DMA dependencies via PE program
            # order instead of carrying their own sync waits (the LDW
            # instruction has very few wait slots).
            warm = psump.tile([128, 4, C], F32, tag="ps0")
            # PE clock rampers: keep the PE continuously busy from t~0
            # (dependent only on the ones memset) so it reaches full
            # p-state before the first weight-dependent matmuls.
            for _ in range(8):
                nc.tensor.matmul(warm[:, 0, :], lhsT=ones_sb[:, 0:128],
                                 rhs=ones_sb, start=True, stop=True,
                                 skip_group_check=True)
            nc.tensor.matmul(warm[:, 0, :], lhsT=whh_sb[:, 0:128],
                             rhs=whh_sb[:, 0:C], start=True, stop=True,
                             skip_group_check=True)
            nc.tensor.matmul(warm[:, 0, :], lhsT=whh_neg[:, 0:128],
                             rhs=whh_neg[:, 0:C], start=True, stop=True,
                             skip_group_check=True)
            nc.tensor.matmul(warm[:, 0, :], lhsT=wih_sb[:, 0, 0:128],
                             rhs=wih_sb[:, 1, 0:C], start=True, stop=True,
                             skip_group_check=True)
            nc.tensor.matmul(warm[:, 0, :], lhsT=bias_sb[:, 0:128],
                             rhs=ones_sb, start=True, stop=True,
                             skip_group_check=True)

            carry_h = []
            carry_c = []   # holds c/2 (the scan accumulates z/2 terms)
            hs_pp = []
            for s in range(S):
                ch = singles.tile([128, BS], F16, tag=f"carryh{s}")
                cc = singles.tile([128, BS], F32, tag=f"carryc{s}")
                nc.vector.memset(ch, 0.0)
                nc.vector.memset(cc, 0.0)
                carry_h.append(ch)
                carry_c.append(cc)
                # persistent shifted-h ping-pong tiles (col 0 = carry)
                hs_pp.append((
                    singles.tile([128, BS, L + 1], F16, tag=f"hsA{s}",
                                 name=f"hsA{s}"),
                    singles.tile([128, BS, L + 1], F16, tag=f"hsB{s}",
                                 name=f"hsB{s}"),
                ))

            xt_r = xt_h[:, :, :].transpose([1, 0, 2]).rearrange(
                "p k (blk u t) -> p k blk u t", blk=NBLK, u=U)
            out_r = out_h[:, :].rearrange("p (blk u t) -> p blk u t",
                                          blk=NBLK, u=U)

            def emit_block(s, blk):
                u0 = s * BS
                # ---- x^T block in ----
                xt_t = work.tile([128, 2, BS, L], BF16, tag=f"xt{s}")
                nc.sync.dma_start(
                    out=xt_t,
                    in_=xt_r[:, :, blk, u0:u0 + BS, :],
                )
                ps = psump.tile([128, 4, C], F32, tag=f"ps{s}")
                # ---- xg = W_ih @ x + b  (per gate chunk) ----
                # start=True clears has_written for the WHOLE bank, so it may
                # only be set on the first matmul touching each PSUM bank;
                # later chunks in the same bank first-write via cleared bits.
                chunks_per_bank = max(1, 512 // C)
                for g in range(4):
                    for k in range(2):
                        nc.tensor.matmul(
                            ps[:, g, :],
                            lhsT=wih_sb[:, k, g * 128:(g + 1) * 128],
                            rhs=xt_t[:, k],
                            start=(k == 0 and g % chunks_per_bank == 0),
                            stop=False, skip_group_check=True,
                        )
                    nc.tensor.matmul(
                        ps[:, g, :],
                        lhsT=bias_sb[:, g * 128:(g + 1) * 128],
                        rhs=ones_sb,
                        start=False, stop=False, skip_group_check=True,
                    )
                # ---- shifted-h guess: [carry | zeros] ----
                hsA, hsB = hs_pp[s]
                nc.gpsimd.memset(hsA[:, :, 1:L + 1], 0.0)
                nc.gpsimd.tensor_copy(out=hsA[:, :, 0], in_=carry_h[s])
                nc.gpsimd.tensor_copy(out=hsB[:, :, 0], in_=carry_h[s])
                hs_prev = hsA

                hs_pprev = None
                for sw in range(K_SWEEPS):
                    last = sw == K_SWEEPS - 1
                    # ---- gates += W_hh @ hs_new - W_hh @ hs_old ----
                    # The negative matmuls' operand (hs from two sweeps ago)
                    # is ready early, so they overlap the previous sweep's
                    # scan/tanh phase; only the positive matmuls sit on the
                    # critical path after the h update.
                    # hsA's initial guess is zero except the carry
                    # columns, so the sweep-1 positive matmuls and sweep-2
                    # negative matmuls only need those BS columns.
                    ps_v = ps.rearrange("p g (u t) -> p g u t", u=BS)
                    if sw > 0:
                        sparse_neg = hs_pprev is hsA and sw == 1
                        for g in range(4):
                            if sparse_neg:
                                nc.tensor.matmul(
                                    ps_v[:, g, :, 0:1],
                                    lhsT=whh_neg[:, g * 128:(g + 1) * 128],
                                    rhs=hs_pprev[:, :, 0:1],
                                    start=False, stop=False,
                                    skip_group_check=True,
                                )
                            else:
                                nc.tensor.matmul(
                                    ps[:, g, :],
                                    lhsT=whh_neg[:, g * 128:(g + 1) * 128],
                                    rhs=hs_pprev[:, :, 0:L],
                                    start=False, stop=False,
                                    skip_group_check=True,
                                )
                    for g in range(4):
                        if sw == 0:
                            nc.tensor.matmul(
                                ps_v[:, g, :, 0:1],
                                lhsT=whh_sb[:, g * 128:(g + 1) * 128],
                                rhs=hs_prev[:, :, 0:1],
                                start=False, stop=False,
                                skip_group_check=True,
                            )
                        else:
                            nc.tensor.matmul(
                                ps[:, g, :],
                                lhsT=whh_sb[:, g * 128:(g + 1) * 128],
                                rhs=hs_prev[:, :, 0:L],
                                start=False, stop=(last and g == 3),
                                skip_group_check=True,
                            )
                    # ---- activations: one sigmoid over all 4 chunks;
                    # chunk 3 holds 2g so tanh(g) = 2*sigmoid(2g) - 1 ----
                    ifo = work.tile([128, 4, C], F16, tag=f"ifo{s}")
                    nc.scalar.activation(out=ifo, in_=ps[:, :, :], func=sig)
                    # ---- z' = (sig(2g) - 0.5) * sig(i) = z/2 ----
                    z = work.tile([128, C], F16, tag=f"z{s}", bufs=4)
                    nc.vector.scalar_tensor_tensor(
                        out=z, in0=ifo[:, 3, :], scalar=0.5, in1=ifo[:, 0, :],
                        op0=sub, op1=mult)
                    # ---- c/2-recurrence scan per sequence (DVE + Pool) ----
                    cfull = work.tile([128, C], F32, tag=f"c{s}", bufs=4)
                    for u in range(BS):
                        nc.vector.tensor_tensor_scan(
                            out=cfull[:, u * L:(u + 1) * L],
                            data0=ifo[:, 1, u * L:(u + 1) * L],
                            data1=z[:, u * L:(u + 1) * L],
                            initial=carry_c[s][:, u:u + 1],
                            op0=mult, op1=add,
                        )
                    # ---- tanh(c) = tanh(2 * c/2) ----
                    tcl = work.tile([128, C], F16, tag=f"tc{s}", bufs=4)
                    nc.scalar.activation(out=tcl, in_=cfull, func=tanh,
                                         scale=2.0)
                    # ---- h = sigmoid(o) * tanh(c) ----
                    o_v = ifo[:, 2, :].rearrange("p (u t) -> p u t", u=BS)
                    tc_v = tcl.rearrange("p (u t) -> p u t", u=BS)
                    if last:
                        out_t = work.tile([128, BS, L], F16, tag=f"out{s}")
                        nc.vector.tensor_mul(out_t, o_v, tc_v)
                        nc.sync.dma_start(
                            out=out_r[:, blk, u0:u0 + BS, :], in_=out_t,
                        )
                        # carries for next block
                        nc.gpsimd.tensor_copy(
                            out=carry_h[s], in_=out_t[:, :, L - 1])
                        nc.gpsimd.tensor_copy(
                            out=carry_c[s],
                            in_=cfull.rearrange("p (u t) -> p u t", u=BS)[:, :, L - 1])
                    else:
                        hs_next = hsB if hs_prev is hsA else hsA
                        nc.vector.tensor_mul(hs_next[:, :, 1:L + 1], o_v, tc_v)
                        hs_pprev = hs_prev
                        hs_prev = hs_next

            for blk in range(NBLK):
                for i in range(S):
                    emit_block((i + blk) % S, blk)

    if not nc.is_finalized():
        nc.finalize()
    return nc


def _get_nc():
    if "nc" not in _NC_CACHE:
        _NC_CACHE["nc"] = _build_nc()
    return _NC_CACHE["nc"]


def _flip_padded(x, lengths):
    t = np.arange(x.shape[1])[None, :]
    Ln = lengths[:, None].astype(np.int64)
    idx = np.where(t < Ln, Ln - 1 - t, t)
    return np.take_along_axis(x, idx[:, :, None], axis=1)


def _pack_weights(W_ih, W_hh, b_ih, b_hh):
    # chunk order (i, f, o, g); the g chunk is pre-scaled by 2 because the
    # kernel computes tanh(g) as 2*sigmoid(2g) - 1 inside the fused sigmoid
    # instruction.
    Wi = W_ih.reshape(4, H, I)[PERM].copy()             # [4,128,256]
    Wi[3] *= 2.0
    wih = np.ascontiguousarray(
        Wi.transpose(2, 0, 1).reshape(2, 128, G4)).astype(ml_dtypes.bfloat16)
    Wh = W_hh.reshape(4, H, H)[PERM].copy()             # [4,128,128]
    Wh[3] *= 2.0
    whh = np.ascontiguousarray(
        Wh.transpose(2, 0, 1).reshape(128, G4)).astype(np.float16)
    b4 = (b_ih + b_hh).reshape(4, H)[PERM].copy()
    b4[3] *= 2.0
    b = b4.reshape(1, G4).astype(np.float32)
    return wih, whh, np.ascontiguousarray(b)


def _pack_x(x_shard):
    # [U, T, I] -> [2, 128, U*T] with cols (blk, u, t) block-major
    a = x_shard.transpose(2, 0, 1)                      # [I, U, T]
    a = a.reshape(2, 128, U, NBLK, L).transpose(0, 1, 3, 2, 4)
    return np.ascontiguousarray(
        a.reshape(2, 128, U * T)).astype(ml_dtypes.bfloat16)


def _run(inputs, trace=False):
    x = np.asarray(inputs["x"], np.float32)
    lengths = np.asarray(inputs["lengths"])
    Wf_ih = np.asarray(inputs["Wf_ih"], np.float32)
    Wf_hh = np.asarray(inputs["Wf_hh"], np.float32)
    bf_ih = np.asarray(inputs["bf_ih"], np.float32)
    bf_hh = np.asarray(inputs["bf_hh"], np.float32)
    Wb_ih = np.asarray(inputs["Wb_ih"], np.float32)
    Wb_hh = np.asarray(inputs["Wb_hh"], np.float32)
    bb_ih = np.asarray(inputs["bb_ih"], np.float32)
    bb_hh = np.asarray(inputs["bb_hh"], np.float32)

    x_rev = _flip_padded(x, lengths)
    wf = _pack_weights(Wf_ih, Wf_hh, bf_ih, bf_hh)
    wb = _pack_weights(Wb_ih, Wb_hh, bb_ih, bb_hh)

    in_maps = []
    for c in range(NCORES):
        if c < 4:
            xs = x[c * U:(c + 1) * U]
            wih, whh, b = wf
        else:
            xs = x_rev[(c - 4) * U:(c - 3) * U]
            wih, whh, b = wb
        in_maps.append({
            "xt": _pack_x(xs),
            "wih": wih,
            "whh": whh,
            "bias": b,
        })

    nc = _get_nc()
    res = run_bass_kernel_spmd(nc, in_maps, core_ids=list(range(NCORES)),
                               trace=trace)
    halves = []
    for c in range(NCORES):
        o = np.asarray(res.results[c]["out"], np.float32)
        o = o.reshape(128, NBLK, U, L).transpose(2, 1, 3, 0)
        halves.append(o.reshape(U, T, 128))
    fwd = np.concatenate(halves[0:4], axis=0)   # [32, T, 128]
    bwd = np.concatenate(halves[4:8], axis=0)   # [32, T, 128]
    out = np.ascontiguousarray(
        np.concatenate([fwd, bwd], axis=-1)).astype(np.float32)
    return out, res.exec_time_ns


def kernel(**inputs):
    out, _ = _run(inputs, trace=False)
    return out
